# revision 8
# baseline (speedup 1.0000x reference)
"""Trainium2 Bass kernel: local-attention transformer block (window=128).

Strategy: sequence-parallel across 8 NeuronCores. Each core gets 512 own
tokens plus a 128-token halo on each side (768 local tokens). The
attention window (|i-j| <= 128) never crosses the halo, so there are no
collectives: each core independently computes LN1 -> QKV -> local
attention -> out-proj -> residual -> LN2 -> FFN -> residual for its 512
tokens, and the host concatenates the 8 slices.

On-chip dataflow (per core, P=128 partitions):
  - LN1 stats token-major (bn_stats/bn_aggr), normalized tiles are
    PE-transposed into feature-major xnT [d, tok].
  - Q/K projections: weight-stationary matmuls (float32r) producing
    feature-major QT/KT [feat, tok]; the 1/sqrt(hd) scale is folded into
    the Q PSUM->SBUF copy.
  - V projection: activation-stationary, producing token-major V
    [tok, feat] in bf16 (PV matmul operand).
  - Attention per (head, query-block): scores are computed TRANSPOSED
    ([key, query]) so that after exp the probabilities are already in
    the layout PV needs as the stationary operand: no probability
    transposes. The softmax denominator comes from an extra matmul of
    the probability tile against a ones-vector (same stationary weights
    -> nearly free), yielding [query, 1] per-partition reciprocals; the
    max-subtraction is skipped (scores are O(5) here, exp is safe) and
    masked entries carry an additive -60.
  - out-proj / FFN1 / FFN2 are plain tiled matmuls (f32r except FFN2
    which runs in bf16), with residual adds fused into the PSUM->SBUF
    copies on the vector engine, GELU+bias on the scalar engine.
"""

from contextlib import ExitStack

import numpy as np
import ml_dtypes

import concourse.bass as bass
import concourse.tile as tile
from concourse import bacc, mybir
from concourse.bass_utils import run_bass_kernel_spmd
from concourse.masks import make_identity

T, D, H, HD, MLP, WIN = 4096, 1024, 16, 64, 4096, 128
EPS = 1e-5
NCORES = 8
NTOK = T // NCORES           # 512 own tokens per core
LTOK = NTOK + 2 * WIN        # 768 local tokens (incl. halo)
P = 128
ND = D // P                  # 8  d-chunks
NFQK = 2 * D // P            # 16 q+k feature chunks
NTT = LTOK // P              # 6  local token tiles
NQB = NTOK // P              # 4  query blocks
NMF = MLP // P               # 32 mlp feature chunks
MASKVAL = -60.0
QSCALE = float(HD) ** -0.5

f32 = mybir.dt.float32
f32r = mybir.dt.float32r
bf16 = mybir.dt.bfloat16
AF = mybir.ActivationFunctionType
ALU = mybir.AluOpType


def r(ap):
    """View an fp32 AP as float32r for full-rate PE matmuls."""
    return ap.bitcast(f32r)


def _build_body(nc, tc, ctx, dram, opt_d, flags):
    has = lambda k: k in flags
    x_d, wqk_d, wv_d, wo_d, w1_d, w2_d, mask_d, out_d = dram

    pool = lambda name, bufs, **kw: ctx.enter_context(
        tc.tile_pool(name=name, bufs=bufs, **kw))

    constp = pool("const", 1)
    outp = pool("outp", 4)
    statp = pool("stat", 4)
    tmps = pool("tmps", 4)
    wsp = pool("wstream", 3)
    w2sp = pool("w2stream", 4)
    attnp = pool("attn", 3)
    ps_tr_stack = ctx.enter_context(ExitStack())
    ps_tr = ps_tr_stack.enter_context(
        tc.tile_pool(name="ps_tr", bufs=2, space="PSUM"))

    identity = constp.tile([P, P], f32)
    make_identity(nc, identity)
    eps_t = constp.tile([P, 1], f32)
    nc.vector.memset(eps_t, EPS)
    ones_c = constp.tile([P, 1], bf16)
    nc.vector.memset(ones_c, 1.0)
    maskT = constp.tile([P, NQB, 2, P], f32)
    nc.sync.dma_start(maskT, mask_d.rearrange("a b k q -> k a b q"))

    opt_sb = {}
    for k in ("g1", "be1", "g2", "be2"):
        if has(k):
            opt_sb[k] = constp.tile([P, ND], f32, name=k + "_sb")
            nc.sync.dma_start(opt_sb[k], opt_d[k].rearrange("a p -> p a"))
    if has("bqk"):
        opt_sb["bqk"] = constp.tile([P, NFQK], f32, name="bqk_sb")
        nc.sync.dma_start(opt_sb["bqk"], opt_d["bqk"])
    if has("b1"):
        opt_sb["b1"] = constp.tile([P, NMF], f32, name="b1_sb")
        nc.sync.dma_start(opt_sb["b1"], opt_d["b1"])
    for k in ("bv", "bo", "b2"):
        if has(k):
            opt_sb[k] = constp.tile([P, D], f32, name=k + "_sb")
            src = opt_d[k]
            bcast = bass.AP(tensor=src.tensor, offset=src.offset,
                            ap=[[0, P]] + list(src.ap))
            nc.gpsimd.dma_start(opt_sb[k], bcast)

    def ln_apply_transposed(src_tile, mean, rstd, dst, dst_col0, gk, bek):
        """(src - mean) * rstd per token tile, transposed into dst
        feature-major, with optional per-feature gain/bias."""
        for dc in range(ND):
            tmp = tmps.tile([P, P], f32, tag="lnt")
            nc.vector.tensor_scalar(
                tmp, src_tile[:, dc * P:(dc + 1) * P],
                scalar1=mean, scalar2=rstd,
                op0=ALU.subtract, op1=ALU.mult)
            pst = ps_tr.tile([P, P], f32, tag="trf")
            nc.tensor.transpose(pst, tmp, identity)
            dslc = dst[:, dc, dst_col0:dst_col0 + P]
            if gk is not None or bek is not None:
                g_ap = opt_sb[gk][:, dc:dc + 1] if gk else 1.0
                if bek:
                    nc.vector.scalar_tensor_tensor(
                        dslc, in0=pst, scalar=g_ap,
                        in1=opt_sb[bek][:, dc:dc + 1].to_broadcast((P, P)),
                        op0=ALU.mult, op1=ALU.add)
                else:
                    nc.vector.tensor_scalar_mul(dslc, pst, g_ap)
            else:
                nc.scalar.copy(dslc, pst)

    def ln_stats(src):
        st = statp.tile([P, 2, 6], f32, tag="st")
        for sg in range(2):
            nc.vector.bn_stats(st[:, sg, :], src[:, sg * 512:(sg + 1) * 512])
        mv = statp.tile([P, 2], f32, tag="mv")
        nc.vector.bn_aggr(mv, st)
        sq = statp.tile([P, 1], f32, tag="sq")
        nc.scalar.activation(sq, mv[:, 1:2], AF.Sqrt, bias=eps_t[:, 0:1])
        rstd = statp.tile([P, 1], f32, tag="rstd")
        nc.vector.reciprocal(rstd, sq)
        return mv[:, 0:1], rstd

    mid = ctx.enter_context(ExitStack())
    xp = mid.enter_context(tc.tile_pool(name="xp", bufs=1))
    x_sb = [xp.tile([P, D], f32, name=f"x{tt}", tag=f"x{tt}") for tt in range(NTT)]

    qkv_stack = mid.enter_context(ExitStack())
    xnTp = qkv_stack.enter_context(tc.tile_pool(name="xnTp", bufs=1))
    xnT = xnTp.tile([P, ND, LTOK], f32r)

    # ---- Phase 1+2: load x, LN1, transpose -> xnT
    for tt in range(NTT):
        nc.sync.dma_start(x_sb[tt], x_d[tt * P:(tt + 1) * P, :])
        mean, rstd = ln_stats(x_sb[tt])
        ln_apply_transposed(x_sb[tt], mean, rstd, xnT, tt * P,
                            "g1" if has("g1") else None,
                            "be1" if has("be1") else None)

    # ---- Phase 3: Q/K projections (weight-stationary, feature-major out)
    qkvp = qkv_stack.enter_context(tc.tile_pool(name="qkvp", bufs=1))
    QT = qkvp.tile([P, ND, LTOK], f32r, name="QT")
    KT = qkvp.tile([P, ND, LTOK], f32r, name="KT")
    ps_mm1_stack = ExitStack()
    ps_mm = ps_mm1_stack.enter_context(
        tc.tile_pool(name="ps_mm1", bufs=2, space="PSUM"))
    for fc in range(NFQK):
        wqk_sb = wsp.tile([P, ND, P], f32r, tag="wqk")
        nc.sync.dma_start(wqk_sb, wqk_d[fc])
        ps0 = ps_mm.tile([P, 512], f32, tag="mmA", name="ps0")[:, :384]
        ps1 = ps_mm.tile([P, 512], f32, tag="mmB", name="ps1")[:, :384]
        for dc in range(ND):
            lhsT = wqk_sb[:, dc, :]
            nc.tensor.matmul(ps0, lhsT, xnT[:, dc, 0:384],
                             start=(dc == 0), stop=(dc == ND - 1))
            nc.tensor.matmul(ps1, lhsT, xnT[:, dc, 384:768],
                             start=(dc == 0), stop=(dc == ND - 1))
        if fc < ND:
            dst, scale = QT[:, fc, :], QSCALE
        else:
            dst, scale = KT[:, fc - ND, :], 1.0
        for half, ps in ((0, ps0), (1, ps1)):
            dslc = dst[:, half * 384:(half + 1) * 384]
            if has("bqk"):
                nc.vector.tensor_scalar(
                    dslc, ps, scalar1=opt_sb["bqk"][:, fc:fc + 1],
                    scalar2=scale, op0=ALU.add, op1=ALU.mult)
            else:
                nc.scalar.mul(dslc, ps, scale)

    # ---- Phase 4: V projection (activation-stationary, token-major bf16)
    V = qkvp.tile([P, NTT, D], bf16, name="V")
    wvp = qkv_stack.enter_context(tc.tile_pool(name="wvp", bufs=1))
    wv_sb = wvp.tile([P, ND, D], f32r)
    for dc in range(ND):
        nc.sync.dma_start(wv_sb[:, dc, :], wv_d[dc])
    for tt in range(NTT):
        psA = ps_mm.tile([P, 512], f32, tag="mmA")
        psB = ps_mm.tile([P, 512], f32, tag="mmB")
        for dc in range(ND):
            lhsT = xnT[:, dc, tt * P:(tt + 1) * P]
            nc.tensor.matmul(psA, lhsT, wv_sb[:, dc, 0:512],
                             start=(dc == 0), stop=(dc == ND - 1))
            nc.tensor.matmul(psB, lhsT, wv_sb[:, dc, 512:1024],
                             start=(dc == 0), stop=(dc == ND - 1))
        for half, ps in ((0, psA), (1, psB)):
            dslc = V[:, tt, half * 512:(half + 1) * 512]
            if has("bv"):
                nc.vector.scalar_tensor_tensor(
                    dslc, in0=ps, scalar=1.0,
                    in1=opt_sb["bv"][:, half * 512:(half + 1) * 512],
                    op0=ALU.mult, op1=ALU.add)
            else:
                nc.scalar.copy(dslc, ps)

    ps_mm1_stack.close()

    # ---- Phase 5: attention (transposed scores)
    o_stack = ExitStack()
    op_ = o_stack.enter_context(tc.tile_pool(name="op", bufs=1, side="right"))
    o_sb = op_.tile([P, NQB, D], f32)
    ps_att_stack = ExitStack()
    ps_att = ps_att_stack.enter_context(
        tc.tile_pool(name="ps_att", bufs=2, space="PSUM"))
    for h in range(H):
        fc, po = h // 2, (h % 2) * 64
        for qb in range(NQB):
            ps_s = ps_att.tile([P, 3, P], f32, tag="sT")
            q_rhs = QT[po:po + 64, fc, WIN + qb * P:WIN + (qb + 1) * P]
            for c in range(3):
                nc.tensor.matmul(
                    ps_s[:, c, :],
                    KT[po:po + 64, fc, (qb + c) * P:(qb + c + 1) * P],
                    q_rhs, start=True, stop=True)
            nc.vector.tensor_add(ps_s[:, 0, :], ps_s[:, 0, :], maskT[:, qb, 0, :])
            nc.vector.tensor_add(ps_s[:, 2, :], ps_s[:, 2, :], maskT[:, qb, 1, :])
            pT = attnp.tile([P, 3, P], bf16, tag="pT")
            nc.scalar.activation(pT, ps_s, AF.Exp)
            ps_o = ps_att.tile([P, HD], f32, tag="o")
            ps_dn = ps_att.tile([P, 1], f32, tag="d")
            for c in range(3):
                lhsT = pT[:, c, :]
                nc.tensor.matmul(ps_o, lhsT, V[:, qb + c, h * HD:(h + 1) * HD],
                                 start=(c == 0), stop=(c == 2))
                nc.tensor.matmul(ps_dn, lhsT, ones_c,
                                 start=(c == 0), stop=(c == 2))
            rec = attnp.tile([P, 1], f32, tag="rec")
            nc.vector.reciprocal(rec, ps_dn)
            nc.vector.tensor_scalar_mul(o_sb[:, qb, h * HD:(h + 1) * HD],
                                        ps_o, rec)

    ps_att_stack.close()
    qkv_stack.close()  # QT/KT/V/wv no longer needed

    # ---- Phase 6: transpose o -> oT (feature-major)
    oTp = mid.enter_context(tc.tile_pool(name="oTp", bufs=1))
    oT_sb = oTp.tile([P, ND, NTOK], f32r)
    for qb in range(NQB):
        for dc in range(ND):
            pst = ps_tr.tile([P, P], f32, tag="trf")
            nc.tensor.transpose(pst, o_sb[:, qb, dc * P:(dc + 1) * P], identity)
            nc.scalar.copy(oT_sb[:, dc, qb * P:(qb + 1) * P], pst)

    o_stack.close()  # o transposed; free right-side space for x2

    # ---- Phase 7: out-proj + residual -> x2 (token-major)
    ps_mm2_stack = ExitStack()
    ps_mm = ps_mm2_stack.enter_context(
        tc.tile_pool(name="ps_mm2", bufs=2, space="PSUM"))
    wop = mid.enter_context(tc.tile_pool(name="wop", bufs=1))
    wo_sb = wop.tile([P, ND, D], f32r)
    for dc in range(ND):
        nc.sync.dma_start(wo_sb[:, dc, :], wo_d[dc])
    x2p = ctx.enter_context(tc.tile_pool(name="x2p", bufs=1, side="right"))
    x2 = x2p.tile([P, NQB, D], f32)
    for tt4 in range(NQB):
        psA = ps_mm.tile([P, 512], f32, tag="mmA")
        psB = ps_mm.tile([P, 512], f32, tag="mmB")
        for dc in range(ND):
            lhsT = oT_sb[:, dc, tt4 * P:(tt4 + 1) * P]
            nc.tensor.matmul(psA, lhsT, wo_sb[:, dc, 0:512],
                             start=(dc == 0), stop=(dc == ND - 1))
            nc.tensor.matmul(psB, lhsT, wo_sb[:, dc, 512:1024],
                             start=(dc == 0), stop=(dc == ND - 1))
        for half, ps in ((0, psA), (1, psB)):
            dslc = x2[:, tt4, half * 512:(half + 1) * 512]
            nc.vector.scalar_tensor_tensor(
                dslc, in0=ps, scalar=1.0,
                in1=x_sb[tt4 + 1][:, half * 512:(half + 1) * 512],
                op0=ALU.mult, op1=ALU.add)
            if has("bo"):
                nc.vector.tensor_add(
                    dslc, dslc, opt_sb["bo"][:, half * 512:(half + 1) * 512])

    ps_mm2_stack.close()
    mid.close()  # x, o, oT, wo no longer needed

    # ---- Phase 8: LN2, transpose -> xn2T
    xn2Tp = ctx.enter_context(tc.tile_pool(name="xn2Tp", bufs=1))
    xn2T = xn2Tp.tile([P, ND, NTOK], f32r)
    for tt4 in range(NQB):
        mean, rstd = ln_stats(x2[:, tt4, :])
        ln_apply_transposed(x2[:, tt4, :], mean, rstd, xn2T, tt4 * P,
                            "g2" if has("g2") else None,
                            "be2" if has("be2") else None)

    ps_tr_stack.close()

    # ---- Phase 9: FFN1 (weight-stationary) + GELU -> hT (bf16)
    ps_mm = ctx.enter_context(tc.tile_pool(name="ps_mm3", bufs=2, space="PSUM"))
    hTp = ctx.enter_context(tc.tile_pool(name="hTp", bufs=1))
    hT = hTp.tile([P, NMF, NTOK], bf16)
    for mf in range(NMF):
        w1_sb = wsp.tile([P, ND, P], f32r, tag="wqk")
        nc.sync.dma_start(w1_sb, w1_d[mf])
        ps = ps_mm.tile([P, 512], f32, tag="mmA")
        for dc in range(ND):
            nc.tensor.matmul(ps, w1_sb[:, dc, :], xn2T[:, dc, :],
                             start=(dc == 0), stop=(dc == ND - 1))
        bias = opt_sb["b1"][:, mf:mf + 1] if has("b1") else 0.0
        nc.scalar.activation(hT[:, mf, :], ps, AF.Gelu, bias=bias)

    # ---- Phase 10: FFN2 (bf16) + residual -> out
    ps_f2 = ctx.enter_context(tc.tile_pool(name="ps_f2", bufs=1, space="PSUM"))
    for dh in range(2):
        ps_f = [ps_f2.tile([P, 512], f32, tag=f"f{tt4}", name=f"psf{tt4}")
                for tt4 in range(NQB)]
        for mc in range(NMF):
            w2_sb = w2sp.tile([P, 512], bf16, tag="w2")
            nc.sync.dma_start(w2_sb, w2_d[dh, mc])
            for tt4 in range(NQB):
                nc.tensor.matmul(ps_f[tt4], hT[:, mc, tt4 * P:(tt4 + 1) * P],
                                 w2_sb, start=(mc == 0), stop=(mc == NMF - 1))
        for tt4 in range(NQB):
            y = outp.tile([P, 512], f32, tag="y")
            nc.vector.scalar_tensor_tensor(
                y, in0=ps_f[tt4], scalar=1.0,
                in1=x2[:, tt4, dh * 512:(dh + 1) * 512],
                op0=ALU.mult, op1=ALU.add)
            if has("b2"):
                nc.vector.tensor_add(y, y, opt_sb["b2"][:, dh * 512:(dh + 1) * 512])
            nc.sync.dma_start(
                out_d[tt4 * P:(tt4 + 1) * P, dh * 512:(dh + 1) * 512], y)


def _build_program(flags: frozenset):
    """Build + compile the SPMD single-core program. `flags` enables the
    general paths for non-zero biases / non-unit gains."""
    nc = bacc.Bacc("TRN2", target_bir_lowering=False, debug=False,
                   num_devices=NCORES)

    x_d = nc.dram_tensor("x_t", [LTOK, D], f32, kind="ExternalInput").ap()
    wqk_d = nc.dram_tensor("wqk_pt", [NFQK, P, ND, P], f32r, kind="ExternalInput").ap()
    wv_d = nc.dram_tensor("wv_pt", [ND, P, D], f32r, kind="ExternalInput").ap()
    wo_d = nc.dram_tensor("wo_pt", [ND, P, D], f32r, kind="ExternalInput").ap()
    w1_d = nc.dram_tensor("w1_pt", [NMF, P, ND, P], f32r, kind="ExternalInput").ap()
    w2_d = nc.dram_tensor("w2_pt", [2, NMF, P, 512], bf16, kind="ExternalInput").ap()
    mask_d = nc.dram_tensor("maskT", [NQB, 2, P, P], f32, kind="ExternalInput").ap()
    out_d = nc.dram_tensor("out", [NTOK, D], f32, kind="ExternalOutput").ap()

    opt_d = {}
    for k, shape in (
        ("g1", [ND, P]), ("be1", [ND, P]), ("g2", [ND, P]), ("be2", [ND, P]),
        ("bqk", [P, NFQK]), ("bv", [D]), ("bo", [D]), ("b1", [P, NMF]), ("b2", [D]),
    ):
        if k in flags:
            opt_d[k] = nc.dram_tensor(k + "_in", shape, f32,
                                      kind="ExternalInput").ap()

    dram = (x_d, wqk_d, wv_d, wo_d, w1_d, w2_d, mask_d, out_d)
    with tile.TileContext(nc) as tc:
        with ExitStack() as ctx:
            _build_body(nc, tc, ctx, dram, opt_d, flags)

    nc.compile()
    return nc


_programs = {}


def _get_program(flags: frozenset):
    if flags not in _programs:
        _programs[flags] = _build_program(flags)
    return _programs[flags]


def _mask_for_core(c: int) -> np.ndarray:
    """Additive transposed masks [qb, ci, key, query] for chunks 0 and 2
    of each query block's 384-key window (chunk 1 is always fully valid)."""
    m = np.empty((NQB, 2, P, P), np.float32)
    qo = np.arange(P)[None, :]
    ko = np.arange(P)[:, None]
    for qb in range(NQB):
        for ci, c2 in ((0, 0), (1, 2)):
            gq = c * NTOK + qb * P + qo
            gk = c * NTOK - WIN + qb * P + c2 * P + ko
            valid = (np.abs(gq - gk) <= WIN) & (gk >= 0) & (gk < T)
            m[qb, ci] = np.where(valid, 0.0, MASKVAL)
    return m


def _prep_host(inputs):
    x = np.asarray(inputs["x"], np.float32)
    w_qkv = np.asarray(inputs["w_qkv"], np.float32)
    w_out = np.asarray(inputs["w_out"], np.float32)
    w1 = np.asarray(inputs["w1"], np.float32)
    w2 = np.asarray(inputs["w2"], np.float32)

    w_q, w_k, w_v = w_qkv[:D], w_qkv[D:2 * D], w_qkv[2 * D:]
    wqk_pt = np.ascontiguousarray(
        np.concatenate([w_q, w_k], 0).reshape(NFQK, P, ND, P).transpose(0, 3, 2, 1))
    wv_pt = np.ascontiguousarray(w_v.T.reshape(ND, P, D))
    wo_pt = np.ascontiguousarray(w_out.T.reshape(ND, P, D))
    w1_pt = np.ascontiguousarray(w1.reshape(NMF, P, ND, P).transpose(0, 3, 2, 1))
    w2_pt = np.ascontiguousarray(
        w2.reshape(2, 512, NMF, P).transpose(0, 2, 3, 1)).astype(ml_dtypes.bfloat16)

    x_pad = np.zeros((T + 2 * WIN, D), np.float32)
    x_pad[WIN:WIN + T] = x.reshape(T, D)

    shared = {
        "wqk_pt": wqk_pt, "wv_pt": wv_pt, "wo_pt": wo_pt,
        "w1_pt": w1_pt, "w2_pt": w2_pt,
    }

    flags = set()

    def _nontrivial(k, arr, default):
        arr = np.asarray(arr, np.float32)
        if np.allclose(arr, default):
            return None
        flags.add(k)
        return arr

    g1 = _nontrivial("g1", inputs["g1"], 1.0)
    be1 = _nontrivial("be1", inputs["be1"], 0.0)
    g2 = _nontrivial("g2", inputs["g2"], 1.0)
    be2 = _nontrivial("be2", inputs["be2"], 0.0)
    for k, v in (("g1", g1), ("be1", be1), ("g2", g2), ("be2", be2)):
        if v is not None:
            shared[k + "_in"] = np.ascontiguousarray(v.reshape(ND, P))
    bqkv = np.asarray(inputs["b_qkv"], np.float32)
    if np.any(bqkv[:2 * D]):
        flags.add("bqk")
        shared["bqk_in"] = np.ascontiguousarray(bqkv[:2 * D].reshape(NFQK, P).T)
    if np.any(bqkv[2 * D:]):
        flags.add("bv")
        shared["bv_in"] = np.ascontiguousarray(bqkv[2 * D:])
    for k, key in (("bo", "b_out"), ("b1", "b1"), ("b2", "b2")):
        arr = np.asarray(inputs[key], np.float32)
        if np.any(arr):
            flags.add(k)
            if k == "b1":
                shared["b1_in"] = np.ascontiguousarray(arr.reshape(NMF, P).T)
            else:
                shared[k + "_in"] = np.ascontiguousarray(arr)

    in_maps = []
    for c in range(NCORES):
        m = dict(shared)
        m["x_t"] = np.ascontiguousarray(x_pad[c * NTOK:c * NTOK + LTOK])
        m["maskT"] = _mask_for_core(c)
        in_maps.append(m)
    return frozenset(flags), in_maps


def kernel(**inputs) -> np.ndarray:
    flags, in_maps = _prep_host(inputs)
    nc = _get_program(flags)
    res = run_bass_kernel_spmd(nc, in_maps, core_ids=list(range(NCORES)))
    out = np.concatenate([res.results[c]["out"] for c in range(NCORES)], axis=0)
    return out.reshape(1, T, D).astype(np.float32)


# revision 24
# speedup vs baseline: 1.3036x; 1.3036x over previous
"""Trainium2 Bass kernel: local-attention transformer block (window=128).

Strategy: sequence-parallel across 8 NeuronCores. Each core gets 512 own
tokens plus a 128-token halo on each side (768 local tokens). The
attention window (|i-j| <= 128) never crosses the halo, so there are no
collectives: each core independently computes LN1 -> QKV -> local
attention -> out-proj -> residual -> LN2 -> FFN -> residual for its 512
tokens, and the host concatenates the 8 slices.

Per-core dataflow (P=128 partitions):
  - LN1 stats token-major (bn_stats/bn_aggr); normalized tiles are
    PE-transposed into feature-major xnT [d, tok] (bf16).
  - Q/K projections: weight-stationary bf16 matmuls producing
    feature-major QT/KT [feat, tok]; Q only covers the 512 own tokens
    and carries the 1/sqrt(hd) scale folded into its PSUM->SBUF copy.
  - V projection: activation-stationary, token-major V in bf16, stored
    per-head with an extra ones-column so the PV matmul produces the
    softmax denominator for free.
  - Attention (query-block outer, head inner): scores are computed
    TRANSPOSED ([key, query]) so the exp'd probabilities are already in
    the layout PV needs as the stationary operand - no probability
    transposes. Masking is a binary multiply on the GPSIMD engine
    (otherwise idle), keeping the vector engine off the critical path.
    Max-subtraction is skipped (scores are O(5), exp is safe).
  - The ACT-engine-bound attention region is backfilled with PE work:
    after each query block finishes, its o-transposes, out-projection
    (+residual into x2) and LN2 (+transposes into xn2T) are emitted so
    they interleave with the next query block's softmax.
  - FFN1 (weight-stationary, GELU on ACT) -> hT bf16; FFN2 accumulates
    4 token-tiles across all 32 mlp chunks per output half, with the
    residual fused into the PSUM->SBUF copy.
  All matmul inputs are bf16 (accumulation is fp32 in PSUM); LayerNorm
  statistics, residuals and softmax denominators stay fp32.
"""

from contextlib import ExitStack

import numpy as np
import ml_dtypes

import concourse.bass as bass
import concourse.tile as tile
from concourse import bacc, mybir
from concourse.bass_utils import run_bass_kernel_spmd
from concourse.masks import make_identity

T, D, H, HD, MLP, WIN = 4096, 1024, 16, 64, 4096, 128
EPS = 1e-5
NCORES = 8
NTOK = T // NCORES           # 512 own tokens per core
LTOK = NTOK + 2 * WIN        # 768 local tokens (incl. halo)
P = 128
ND = D // P                  # 8  d-chunks
NFQK = 2 * D // P            # 16 q+k feature chunks
NTT = LTOK // P              # 6  local token tiles
NQB = NTOK // P              # 4  query blocks
NMF = MLP // P               # 32 mlp feature chunks
QSCALE = float(HD) ** -0.5

f32 = mybir.dt.float32
bf16 = mybir.dt.bfloat16
AF = mybir.ActivationFunctionType
ALU = mybir.AluOpType


def _build_body(nc, tc, ctx, dram, opt_d, flags):
    has = lambda k: k in flags
    x_d, wqk_d, wv_d, wo_d, w1_d, w2_d, mask_d, out_d = dram

    pool = lambda name, bufs, **kw: ctx.enter_context(
        tc.tile_pool(name=name, bufs=bufs, **kw))

    constp = pool("const", 1)
    outp = pool("outp", 4)
    statp = pool("stat", 4)
    tmps = pool("tmps", 4)
    wsp = pool("wstream", 3)
    w2sp = pool("w2stream", 4)
    attnp = pool("attn", 6)
    psum = pool("psum", 2, space="PSUM")   # tags: trf, mmA, mmB, sT, od

    identity = constp.tile([P, P], bf16)
    make_identity(nc, identity)
    eps_t = constp.tile([P, 1], f32)
    nc.vector.memset(eps_t, EPS)
    # touch the Sqrt act table first so its load overlaps the initial DMAs
    warm = constp.tile([P, 1], f32)
    nc.scalar.activation(warm, eps_t, AF.Sqrt)
    maskT = constp.tile([P, NQB, 2, P], bf16)
    nc.gpsimd.dma_start(maskT, mask_d.rearrange("a b k q -> k a b q"))

    opt_sb = {}
    for k in ("g1", "be1", "g2", "be2"):
        if has(k):
            opt_sb[k] = constp.tile([P, ND], f32, name=k + "_sb")
            nc.sync.dma_start(opt_sb[k], opt_d[k].rearrange("a p -> p a"))
    if has("bqk"):
        opt_sb["bqk"] = constp.tile([P, NFQK], f32, name="bqk_sb")
        nc.sync.dma_start(opt_sb["bqk"], opt_d["bqk"])
    if has("b1"):
        opt_sb["b1"] = constp.tile([P, NMF], f32, name="b1_sb")
        nc.sync.dma_start(opt_sb["b1"], opt_d["b1"])
    for k in ("bv", "bo", "b2"):
        if has(k):
            opt_sb[k] = constp.tile([P, D], f32, name=k + "_sb")
            src = opt_d[k]
            bcast = bass.AP(tensor=src.tensor, offset=src.offset,
                            ap=[[0, P]] + list(src.ap))
            nc.gpsimd.dma_start(opt_sb[k], bcast)

    def ln_stats(src, newton):
        st = statp.tile([P, 2, 6], f32, tag="st")
        for sg in range(2):
            nc.vector.bn_stats(st[:, sg, :], src[:, sg * 512:(sg + 1) * 512])
        mv = statp.tile([P, 2], f32, tag="mv")
        nc.vector.bn_aggr(mv, st)
        rstd = statp.tile([P, 1], f32, tag="rstd")
        if newton:
            # DVE-only rsqrt: keeps ACT on the Exp table mid-attention
            ve = statp.tile([P, 1], f32, tag="ve")
            nc.vector.tensor_scalar_add(ve, mv[:, 1:2], EPS)
            ri = rstd.bitcast(mybir.dt.int32)
            nc.vector.tensor_scalar(ri, ve.bitcast(mybir.dt.int32),
                                    scalar1=1, scalar2=None,
                                    op0=ALU.arith_shift_right)
            nc.vector.tensor_scalar(ri, ri, scalar1=0x5F3759DF, scalar2=-1,
                                    op0=ALU.subtract, op1=ALU.mult)
            t_a = statp.tile([P, 1], f32, tag="t_a")
            for _ in range(2):
                nc.vector.tensor_tensor(t_a, rstd, rstd, op=ALU.mult)
                nc.vector.tensor_tensor(t_a, t_a, ve, op=ALU.mult)
                nc.vector.tensor_scalar(t_a, t_a, scalar1=-0.5, scalar2=1.5,
                                        op0=ALU.mult, op1=ALU.add)
                nc.vector.tensor_tensor(rstd, rstd, t_a, op=ALU.mult)
        else:
            sq = statp.tile([P, 1], f32, tag="sq")
            nc.scalar.activation(sq, mv[:, 1:2], AF.Sqrt, bias=eps_t[:, 0:1])
            nc.vector.reciprocal(rstd, sq)
        nmr = statp.tile([P, 1], f32, tag="nmr")
        nc.vector.scalar_tensor_tensor(nmr, in0=mv[:, 0:1], scalar=-1.0,
                                       in1=rstd, op0=ALU.mult, op1=ALU.mult)
        return nmr, rstd

    def ln_apply_transposed(src_tile, nmr, rstd, dst, dst_col0, gk, bek,
                            copies_on_act):
        """(src * rstd + (-mean*rstd)) as one ACT op, bf16, then PE-transposed
        into feature-major dst; optional per-feature gain/bias fused into
        the PSUM->SBUF copy."""
        tmp = tmps.tile([P, D], bf16, tag="lnf")
        nc.scalar.activation(tmp, src_tile, AF.Identity, bias=nmr, scale=rstd)
        for dc in range(ND):
            pst = psum.tile([P, P], bf16, tag="trf", bufs=2, name="pst")
            nc.tensor.transpose(pst, tmp[:, dc * P:(dc + 1) * P], identity)
            dslc = dst[:, dc, dst_col0:dst_col0 + P]
            if gk is not None or bek is not None:
                g_ap = opt_sb[gk][:, dc:dc + 1] if gk else 1.0
                if bek:
                    nc.vector.scalar_tensor_tensor(
                        dslc, in0=pst, scalar=g_ap,
                        in1=opt_sb[bek][:, dc:dc + 1].to_broadcast((P, P)),
                        op0=ALU.mult, op1=ALU.add)
                else:
                    nc.vector.tensor_scalar_mul(dslc, pst, g_ap)
            elif copies_on_act:
                nc.scalar.copy(dslc, pst)
            else:
                nc.vector.tensor_copy(dslc, pst)

    mid = ctx.enter_context(ExitStack())
    oTp = mid.enter_context(tc.tile_pool(name="oTp", bufs=1))
    wop = mid.enter_context(tc.tile_pool(name="wop", bufs=1))
    xp = mid.enter_context(tc.tile_pool(name="xp", bufs=1))
    oT_sb = oTp.tile([P, ND, NTOK], bf16)
    x_sb = [xp.tile([P, D], f32, name=f"x{tt}", tag=f"x{tt}") for tt in range(NTT)]

    x2p = ctx.enter_context(tc.tile_pool(name="x2p", bufs=1, side="right"))
    x2 = x2p.tile([P, NQB, D], f32)
    xn2Tp = ctx.enter_context(tc.tile_pool(name="xn2Tp", bufs=1, side="right"))
    xn2T = xn2Tp.tile([P, ND, NTOK], bf16)

    qkv_stack = mid.enter_context(ExitStack())
    qkvp = qkv_stack.enter_context(tc.tile_pool(name="qkvp", bufs=1))
    QT = qkvp.tile([P, ND, LTOK], bf16, name="QT")
    KT = qkvp.tile([P, ND, LTOK], bf16, name="KT")
    V = qkvp.tile([P, NTT, H, HD + 1], bf16, name="V")
    nc.vector.memset(V[:, :, :, HD:HD + 1], 1.0)

    inner_stack = qkv_stack.enter_context(ExitStack())
    xnTp = inner_stack.enter_context(tc.tile_pool(name="xnTp", bufs=1))
    xnT = xnTp.tile([P, ND, LTOK], bf16)

    # ---- Phase 1+2: load x, LN1, transpose -> xnT (bf16)
    for tt in range(NTT):
        nc.sync.dma_start(x_sb[tt], x_d[tt * P:(tt + 1) * P, :])
        nmr, rstd = ln_stats(x_sb[tt], newton=False)
        ln_apply_transposed(x_sb[tt], nmr, rstd, xnT, tt * P,
                            "g1" if has("g1") else None,
                            "be1" if has("be1") else None,
                            copies_on_act=True)

    # ---- Phase 3: Q/K projections (weight-stationary, feature-major out)
    for fc in range(NFQK):
        wqk_sb = wsp.tile([P, ND, P], bf16, tag="wqk")
        nc.sync.dma_start(wqk_sb, wqk_d[fc])
        if fc < ND:
            # Q: only the 512 own tokens (halo tokens are never queries)
            ps0 = psum.tile([P, 512], f32, tag="mm", bufs=3, name="ps0")
            for dc in range(ND):
                nc.tensor.matmul(ps0, wqk_sb[:, dc, :], xnT[:, dc, WIN:WIN + NTOK],
                                 start=(dc == 0), stop=(dc == ND - 1))
            dslc = QT[:, fc, WIN:WIN + NTOK]
            if has("bqk"):
                nc.vector.tensor_scalar(
                    dslc, ps0, scalar1=opt_sb["bqk"][:, fc:fc + 1],
                    scalar2=QSCALE, op0=ALU.add, op1=ALU.mult)
            else:
                nc.vector.tensor_scalar_mul(dslc, ps0, QSCALE)
        else:
            ps0 = psum.tile([P, 512], f32, tag="mm", bufs=3, name="ps0")[:, :384]
            ps1 = psum.tile([P, 512], f32, tag="mm", bufs=3, name="ps1")[:, :384]
            for dc in range(ND):
                lhsT = wqk_sb[:, dc, :]
                nc.tensor.matmul(ps0, lhsT, xnT[:, dc, 0:384],
                                 start=(dc == 0), stop=(dc == ND - 1))
                nc.tensor.matmul(ps1, lhsT, xnT[:, dc, 384:768],
                                 start=(dc == 0), stop=(dc == ND - 1))
            dst = KT[:, fc - ND, :]
            for half, ps in ((0, ps0), (1, ps1)):
                dslc = dst[:, half * 384:(half + 1) * 384]
                if has("bqk"):
                    nc.vector.tensor_scalar(
                        dslc, ps, scalar1=opt_sb["bqk"][:, fc:fc + 1],
                        scalar2=1.0, op0=ALU.add, op1=ALU.mult)
                else:
                    nc.vector.tensor_copy(dslc, ps)

    # ---- Phase 4: V projection (activation-stationary, token-major bf16,
    # per-head layout with a trailing ones column for the denominator)
    wvp = inner_stack.enter_context(tc.tile_pool(name="wvp", bufs=1))
    wv_sb = wvp.tile([P, ND, D], bf16)
    for dc in range(ND):
        nc.sync.dma_start(wv_sb[:, dc, :], wv_d[dc])
    for tt in range(NTT):
        psA = psum.tile([P, 512], f32, tag="mm", bufs=3, name="psA")
        psB = psum.tile([P, 512], f32, tag="mm", bufs=3, name="psB")
        for dc in range(ND):
            lhsT = xnT[:, dc, tt * P:(tt + 1) * P]
            nc.tensor.matmul(psA, lhsT, wv_sb[:, dc, 0:512],
                             start=(dc == 0), stop=(dc == ND - 1))
            nc.tensor.matmul(psB, lhsT, wv_sb[:, dc, 512:1024],
                             start=(dc == 0), stop=(dc == ND - 1))
        for half, ps in ((0, psA), (1, psB)):
            dslc = V[:, tt, half * 8:(half + 1) * 8, 0:HD]
            src_v = ps.rearrange("p (h c) -> p h c", h=8)
            if has("bv"):
                nc.vector.scalar_tensor_tensor(
                    dslc, in0=src_v, scalar=1.0,
                    in1=opt_sb["bv"][:, half * 512:(half + 1) * 512].rearrange(
                        "p (h c) -> p h c", h=8),
                    op0=ALU.mult, op1=ALU.add)
            else:
                nc.vector.tensor_copy(dslc, src_v)

    inner_stack.close()  # free xnT + wv before the attention region

    # wo needed mid-attention: stream it in now on the gpsimd queue
    wo_sb = wop.tile([P, ND, D], bf16)
    for dc in range(ND):
        nc.sync.dma_start(wo_sb[:, dc, :], wo_d[dc])

    # ---- Phase 5-8 fused: attention (qb outer), then per query block:
    # o-transposes, out-proj + residual -> x2, LN2 -> xn2T. The trailing
    # per-block work backfills the PE while ACT runs the next block's exps.
    def attn_block(qb):
        o_qb = attnp.tile([P, D], bf16, tag="o_qb", bufs=2, name=f"o{qb}")
        for hg in range(H // 4):
            ps_od = psum.tile([P, 4, HD + 1], f32, tag="od", bufs=1,
                              name="ps_od")
            for u in range(4):
                h = hg * 4 + u
                fc, po = h // 2, (h % 2) * 64
                ps_s = psum.tile([P, 3, P], f32, tag="sT", bufs=2, name="ps_s")
                q_rhs = QT[po:po + 64, fc, WIN + qb * P:WIN + (qb + 1) * P]
                for c in range(3):
                    nc.tensor.matmul(
                        ps_s[:, c, :],
                        KT[po:po + 64, fc, (qb + c) * P:(qb + c + 1) * P],
                        q_rhs, start=True, stop=True)
                pT = attnp.tile([P, 3, P], bf16, tag="pT")
                nc.scalar.activation(pT, ps_s, AF.Exp)
                # binary mask on chunks 0/2 (bf16 SBUF op, vector engine)
                nc.vector.tensor_tensor(pT[:, 0:3:2, :], pT[:, 0:3:2, :],
                                        maskT[:, qb, :, :], op=ALU.mult)
                # fused attention output + softmax denominator (ones col)
                for c in range(3):
                    nc.tensor.matmul(ps_od[:, u, :], pT[:, c, :],
                                     V[:, qb + c, h, :],
                                     start=(c == 0), stop=(c == 2))
            rec = attnp.tile([P, 4], f32, tag="rec")
            nc.vector.reciprocal(rec, ps_od[:, :, HD])
            nc.vector.tensor_tensor(
                o_qb.rearrange("p (g c) -> p g c", c=HD)[:, hg * 4:(hg + 1) * 4, :],
                ps_od[:, :, 0:HD],
                rec[:, :, None].to_broadcast((P, 4, HD)), op=ALU.mult)
            for dc in (2 * hg, 2 * hg + 1):
                # 8 heads -> 2 o feature chunks ready: transpose them now
                pst = psum.tile([P, P], bf16, tag="trf", bufs=2, name="psto")
                nc.tensor.transpose(pst, o_qb[:, dc * P:(dc + 1) * P], identity)
                nc.vector.tensor_copy(oT_sb[:, dc, qb * P:(qb + 1) * P], pst)

    def outproj_block(qb):
        psA = psum.tile([P, 512], f32, tag="mm", bufs=3, name="psoA")
        psB = psum.tile([P, 512], f32, tag="mm", bufs=3, name="psoB")
        for dc in range(ND):
            lhsT = oT_sb[:, dc, qb * P:(qb + 1) * P]
            nc.tensor.matmul(psA, lhsT, wo_sb[:, dc, 0:512],
                             start=(dc == 0), stop=(dc == ND - 1))
            nc.tensor.matmul(psB, lhsT, wo_sb[:, dc, 512:1024],
                             start=(dc == 0), stop=(dc == ND - 1))
        for half, ps in ((0, psA), (1, psB)):
            dslc = x2[:, qb, half * 512:(half + 1) * 512]
            nc.vector.scalar_tensor_tensor(
                dslc, in0=ps, scalar=1.0,
                in1=x_sb[qb + 1][:, half * 512:(half + 1) * 512],
                op0=ALU.mult, op1=ALU.add)
            if has("bo"):
                nc.vector.tensor_add(
                    dslc, dslc, opt_sb["bo"][:, half * 512:(half + 1) * 512])

    def ln2_block(qb):
        nmr, rstd = ln_stats(x2[:, qb, :], newton=True)
        ln_apply_transposed(x2[:, qb, :], nmr, rstd, xn2T, qb * P,
                            "g2" if has("g2") else None,
                            "be2" if has("be2") else None,
                            copies_on_act=False)

    # software pipeline: ln2(qb-1) is emitted after attention(qb) so its
    # transposes fill the PE while the next block's softmax runs on ACT/DVE
    for qb in range(NQB):
        attn_block(qb)
        outproj_block(qb)
        if qb > 0:
            ln2_block(qb - 1)
    ln2_block(NQB - 1)

    mid.close()  # QT/KT/V, x, wo, oT all done

    # ---- Phase 9: FFN1 (weight-stationary) + GELU -> hT (bf16)
    hTp = ctx.enter_context(tc.tile_pool(name="hTp", bufs=1))
    hT = hTp.tile([P, NMF, NTOK], bf16)
    for mf in range(NMF):
        w1_sb = wsp.tile([P, ND, P], bf16, tag="wqk")
        nc.sync.dma_start(w1_sb, w1_d[mf])
        ps = psum.tile([P, 512], f32, tag="mm", bufs=3, name="psf")
        for dc in range(ND):
            nc.tensor.matmul(ps, w1_sb[:, dc, :], xn2T[:, dc, :],
                             start=(dc == 0), stop=(dc == ND - 1))
        bias = opt_sb["b1"][:, mf:mf + 1] if has("b1") else 0.0
        nc.scalar.activation(hT[:, mf, :], ps, AF.Gelu, bias=bias)

    # ---- Phase 10: FFN2 (bf16) + residual -> out
    f2tags = (("sT", "sT", "od", "trf"), ("mm", "mm", "od", "trf"))
    tagbufs = {"sT": 2, "od": 1, "trf": 2, "mm": 3}
    for dh in range(2):
        ps_f = [psum.tile([P, 512], f32, tag=f2tags[dh][tt4],
                          bufs=tagbufs[f2tags[dh][tt4]],
                          name=f"psf2_{dh}_{tt4}") for tt4 in range(NQB)]
        for mc in range(NMF):
            w2_sb = w2sp.tile([P, 512], bf16, tag="w2")
            nc.sync.dma_start(w2_sb, w2_d[dh, mc])
            for tt4 in range(NQB):
                nc.tensor.matmul(ps_f[tt4], hT[:, mc, tt4 * P:(tt4 + 1) * P],
                                 w2_sb, start=(mc == 0), stop=(mc == NMF - 1))
        for tt4 in range(NQB):
            y = outp.tile([P, 512], f32, tag="y")
            nc.vector.scalar_tensor_tensor(
                y, in0=ps_f[tt4], scalar=1.0,
                in1=x2[:, tt4, dh * 512:(dh + 1) * 512],
                op0=ALU.mult, op1=ALU.add)
            if has("b2"):
                nc.vector.tensor_add(y, y, opt_sb["b2"][:, dh * 512:(dh + 1) * 512])
            nc.sync.dma_start(
                out_d[tt4 * P:(tt4 + 1) * P, dh * 512:(dh + 1) * 512], y)


def _build_program(flags: frozenset):
    """Build + compile the SPMD single-core program. `flags` enables the
    general paths for non-zero biases / non-unit gains."""
    nc = bacc.Bacc("TRN2", target_bir_lowering=False, debug=False,
                   num_devices=NCORES)

    x_d = nc.dram_tensor("x_t", [LTOK, D], f32, kind="ExternalInput").ap()
    wqk_d = nc.dram_tensor("wqk_pt", [NFQK, P, ND, P], bf16, kind="ExternalInput").ap()
    wv_d = nc.dram_tensor("wv_pt", [ND, P, D], bf16, kind="ExternalInput").ap()
    wo_d = nc.dram_tensor("wo_pt", [ND, P, D], bf16, kind="ExternalInput").ap()
    w1_d = nc.dram_tensor("w1_pt", [NMF, P, ND, P], bf16, kind="ExternalInput").ap()
    w2_d = nc.dram_tensor("w2_pt", [2, NMF, P, 512], bf16, kind="ExternalInput").ap()
    mask_d = nc.dram_tensor("maskT", [NQB, 2, P, P], bf16, kind="ExternalInput").ap()
    out_d = nc.dram_tensor("out", [NTOK, D], f32, kind="ExternalOutput").ap()

    opt_d = {}
    for k, shape in (
        ("g1", [ND, P]), ("be1", [ND, P]), ("g2", [ND, P]), ("be2", [ND, P]),
        ("bqk", [P, NFQK]), ("bv", [D]), ("bo", [D]), ("b1", [P, NMF]), ("b2", [D]),
    ):
        if k in flags:
            opt_d[k] = nc.dram_tensor(k + "_in", shape, f32,
                                      kind="ExternalInput").ap()

    dram = (x_d, wqk_d, wv_d, wo_d, w1_d, w2_d, mask_d, out_d)
    with tile.TileContext(nc) as tc:
        with ExitStack() as ctx:
            _build_body(nc, tc, ctx, dram, opt_d, flags)

    nc.compile()
    return nc


_programs = {}


def _get_program(flags: frozenset):
    if flags not in _programs:
        _programs[flags] = _build_program(flags)
    return _programs[flags]


def _mask_for_core(c: int) -> np.ndarray:
    """Binary (1=valid) transposed masks [qb, ci, key, query] for chunks 0
    and 2 of each query block's 384-key window (chunk 1 is always valid)."""
    m = np.empty((NQB, 2, P, P), np.float32)
    qo = np.arange(P)[None, :]
    ko = np.arange(P)[:, None]
    for qb in range(NQB):
        for ci, c2 in ((0, 0), (1, 2)):
            gq = c * NTOK + qb * P + qo
            gk = c * NTOK - WIN + qb * P + c2 * P + ko
            valid = (np.abs(gq - gk) <= WIN) & (gk >= 0) & (gk < T)
            m[qb, ci] = np.where(valid, 1.0, 0.0)
    return m.astype(ml_dtypes.bfloat16)


def _prep_host(inputs):
    x = np.asarray(inputs["x"], np.float32)
    w_qkv = np.asarray(inputs["w_qkv"], np.float32)
    w_out = np.asarray(inputs["w_out"], np.float32)
    w1 = np.asarray(inputs["w1"], np.float32)
    w2 = np.asarray(inputs["w2"], np.float32)
    b16 = ml_dtypes.bfloat16

    w_q, w_k, w_v = w_qkv[:D], w_qkv[D:2 * D], w_qkv[2 * D:]
    wqk_pt = np.ascontiguousarray(
        np.concatenate([w_q, w_k], 0).reshape(NFQK, P, ND, P)
        .transpose(0, 3, 2, 1)).astype(b16)
    wv_pt = np.ascontiguousarray(w_v.T.reshape(ND, P, D)).astype(b16)
    wo_pt = np.ascontiguousarray(w_out.T.reshape(ND, P, D)).astype(b16)
    w1_pt = np.ascontiguousarray(
        w1.reshape(NMF, P, ND, P).transpose(0, 3, 2, 1)).astype(b16)
    w2_pt = np.ascontiguousarray(
        w2.reshape(2, 512, NMF, P).transpose(0, 2, 3, 1)).astype(b16)

    x_pad = np.zeros((T + 2 * WIN, D), np.float32)
    x_pad[WIN:WIN + T] = x.reshape(T, D)

    shared = {
        "wqk_pt": wqk_pt, "wv_pt": wv_pt, "wo_pt": wo_pt,
        "w1_pt": w1_pt, "w2_pt": w2_pt,
    }

    flags = set()

    def _nontrivial(k, arr, default):
        arr = np.asarray(arr, np.float32)
        if np.allclose(arr, default):
            return None
        flags.add(k)
        return arr

    g1 = _nontrivial("g1", inputs["g1"], 1.0)
    be1 = _nontrivial("be1", inputs["be1"], 0.0)
    g2 = _nontrivial("g2", inputs["g2"], 1.0)
    be2 = _nontrivial("be2", inputs["be2"], 0.0)
    for k, v in (("g1", g1), ("be1", be1), ("g2", g2), ("be2", be2)):
        if v is not None:
            shared[k + "_in"] = np.ascontiguousarray(v.reshape(ND, P))
    bqkv = np.asarray(inputs["b_qkv"], np.float32)
    if np.any(bqkv[:2 * D]):
        flags.add("bqk")
        shared["bqk_in"] = np.ascontiguousarray(bqkv[:2 * D].reshape(NFQK, P).T)
    if np.any(bqkv[2 * D:]):
        flags.add("bv")
        shared["bv_in"] = np.ascontiguousarray(bqkv[2 * D:])
    for k, key in (("bo", "b_out"), ("b1", "b1"), ("b2", "b2")):
        arr = np.asarray(inputs[key], np.float32)
        if np.any(arr):
            flags.add(k)
            if k == "b1":
                shared["b1_in"] = np.ascontiguousarray(arr.reshape(NMF, P).T)
            else:
                shared[k + "_in"] = np.ascontiguousarray(arr)

    in_maps = []
    for c in range(NCORES):
        m = dict(shared)
        m["x_t"] = np.ascontiguousarray(x_pad[c * NTOK:c * NTOK + LTOK])
        m["maskT"] = _mask_for_core(c)
        in_maps.append(m)
    return frozenset(flags), in_maps


def kernel(**inputs) -> np.ndarray:
    flags, in_maps = _prep_host(inputs)
    nc = _get_program(flags)
    res = run_bass_kernel_spmd(nc, in_maps, core_ids=list(range(NCORES)))
    out = np.concatenate([res.results[c]["out"] for c in range(NCORES)], axis=0)
    return out.reshape(1, T, D).astype(np.float32)
